# revision 16
# baseline (speedup 1.0000x reference)
"""Trainium2 Bass kernel for nn_Block_3539053052091 (hedgehog-style linear
attention block with ALiBi-decay mask, smeared keys, and sandwich layernorms).

Strategy (8 NeuronCores):
  - heads sharded: core c owns heads {2c, 2c+1} for both batches.
  - the host ships x-hat = LN(x) (pre-normalized, affine folded into the
    weights) in bf16 for the v/p projection and in fp8-e4m3 for the q/k
    projection, which runs as DoubleRow fp8 matmuls (2 k-planes per pass);
    per-column biases are added with DVE ops from partition-broadcast tiles.
  - q-hat / k-til transposes ride the DMA xbar (dma_start_transpose) instead
    of the PE; silu(p) is a single ACT Silu op.
  - the causal decayed attention is a chunked linear attention (chunk = 128
    rows) in bf16 with f32 PSUM; the q-softmax normalizer and the 1/s scales
    cancel through the attention row-normalizer; the reference's +1e-5 eps is
    reproduced exactly by adding 1e-5 * s^2 * zq_i to the denominator.
  - each core ships its po = silu(p) * o head-slice (bf16) through one
    AllToAll; it then computes z = po_rows @ W_out for its 512 rows,
    layernorms, and outputs; the host concatenates the 8 slices.
"""

import numpy as np

import concourse.bass as bass
import concourse.mybir as mybir
import concourse.tile as tile
from concourse import bacc
from concourse.masks import make_identity

f32 = mybir.dt.float32
bf16 = mybir.dt.bfloat16
f8 = mybir.dt.float8e4

N_CORES = 8
B = 2
L = 2048
D_MODEL = 1024
HEADS = 16
EXP = 2
D_EXP = D_MODEL * EXP          # 2048
D_HEAD = D_EXP // HEADS        # 128
HPC = HEADS // N_CORES         # heads per core = 2
C = 128                        # chunk (= row tile) size
ROWS = B * L                   # 4096 flattened rows
NT = ROWS // C                 # 32 row tiles
TPB = L // C                   # 16 tiles per batch
KT = D_MODEL // 128            # 8 contraction tiles
NKT = D_EXP // 128             # 16 k-tiles for the output projection
LN_EPS = 1e-5
RB = ROWS // N_CORES           # 512 rows per core after the exchange
S_QK = 32.0                    # fp8 weight scale for the q/k projection

Act = mybir.ActivationFunctionType
Alu = mybir.AluOpType
DR = mybir.MatmulPerfMode.DoubleRow


def build_kernel(mm_dt: str = "bf16", reps: int = 1, no_collective: bool = False,
                 ring_out: bool = False):
    """ring_out=True: write outputs into a 2-slot ring (out shape [2, ...])
    so kernels with different rep counts have identical external I/O —
    needed for clean repeat-slope timing through the axon tunnel."""
    nc = bacc.Bacc("TRN2", target_bir_lowering=False, debug=False,
                   num_devices=N_CORES)

    # packed fp8 x-hat streams: plane 0 = fp8(xh), 1 = fp8(xh/4), 2 = fp8(16*(xh-x8))
    xq_in = nc.dram_tensor("xq", [NT, 128, 3, KT, C], f8, kind="ExternalInput")
    # packed fp8 v/p weights: plane 0 = fp8(32*W), 1 = fp8(-4*epsA), 2 = fp8(2*W)
    wv8_in = nc.dram_tensor("wv8", [3, D_MODEL, 4 * D_HEAD], f8, kind="ExternalInput")
    wq8_in = nc.dram_tensor("wq8", [D_MODEL, 4 * D_HEAD], f8, kind="ExternalInput")
    bvp_in = nc.dram_tensor("bvp", [4 * D_HEAD], f32, kind="ExternalInput")
    bq_in = nc.dram_tensor("bq", [4 * D_HEAD], f32, kind="ExternalInput")
    wout_in = nc.dram_tensor("wout", [D_EXP, D_MODEL], bf16, kind="ExternalInput")
    outw_in = nc.dram_tensor("outw", [D_MODEL], f32, kind="ExternalInput")
    outb_in = nc.dram_tensor("outb", [D_MODEL], f32, kind="ExternalInput")
    dt_in = nc.dram_tensor("dtmask", [HPC, C, C], f32, kind="ExternalInput")
    lamc_in = nc.dram_tensor("lamc", [HPC * C], bf16, kind="ExternalInput")
    # per-(row, head) constants, shipped as [HPC, C] -> loaded [C, HPC]
    mus_in = nc.dram_tensor("mus", [HPC, C], f32, kind="ExternalInput")
    omsig_in = nc.dram_tensor("omsig", [HPC, C], f32, kind="ExternalInput")
    ratio_in = nc.dram_tensor("ratio", [HPC, C], f32, kind="ExternalInput")
    gamc_in = nc.dram_tensor("gamc", [HPC, C], f32, kind="ExternalInput")
    epss_in = nc.dram_tensor("epss", [HPC, C], f32, kind="ExternalInput")

    n_out = min(reps, 2) if ring_out else reps
    out_ext = nc.dram_tensor("out", [n_out, RB, D_MODEL], f32,
                             kind="ExternalOutput")
    nex = 2 if reps > 1 else 1
    pot_dram = nc.dram_tensor("pot", [nex, N_CORES, RB // C, 128, HPC, C],
                              bf16)
    potex_dram = nc.dram_tensor("potex", [nex, N_CORES, RB // C, 128, HPC, C],
                                bf16)

    def bcast_ap(handle, parts=128):
        ap = handle.ap()
        return bass.AP(tensor=ap.tensor, offset=ap.offset,
                       ap=[[0, parts]] + list(ap.ap))

    with tile.TileContext(nc) as tc:
        with (
            tc.tile_pool(name="const", bufs=1) as cst,
            tc.tile_pool(name="zrp", bufs=1) as zrp,
            tc.tile_pool(name="work", bufs=4) as wk,
            tc.tile_pool(name="work2", bufs=2) as wk2,
            tc.tile_pool(name="x8p", bufs=3) as x8p,
            tc.tile_pool(name="small", bufs=4) as sm,
            tc.tile_pool(name="state", bufs=6) as st,
            tc.tile_pool(name="statp", bufs=1) as sp,
            tc.tile_pool(name="pproj", bufs=4, space="PSUM") as pproj,
            tc.tile_pool(name="ptp", bufs=2, space="PSUM") as ptp,
            tc.tile_pool(name="pacc", bufs=2, space="PSUM") as pacc,
        ):
            # ---- constants ----
            identb = cst.tile([128, 128], bf16)
            make_identity(nc, identb[:])
            # shift matrices for the smeared-key row shift:
            # mshift[j', j'+1] = 1 (out row j <- khat2 row j-1)
            mshift = cst.tile([128, 128], bf16)
            nc.gpsimd.memset(mshift[:], 0.0)
            nc.gpsimd.affine_select(
                out=mshift[:], in_=mshift[:],
                compare_op=Alu.not_equal, fill=1.0, base=1,
                pattern=[[-1, 128]], channel_multiplier=1)
            # ecorner[127, 0] = 1 (out row 0 <- prev tile's row 127)
            ecorner = cst.tile([128, 128], bf16)
            nc.gpsimd.memset(ecorner[:], 0.0)
            nc.gpsimd.affine_select(
                out=ecorner[:], in_=ecorner[:],
                compare_op=Alu.not_equal, fill=1.0, base=-127,
                pattern=[[-1, 128]], channel_multiplier=1)
            eps_t = cst.tile([128, 1], f32)
            nc.vector.memset(eps_t[:], LN_EPS)
            zero_k = cst.tile([128, 2 * D_HEAD], bf16)
            nc.vector.memset(zero_k[:], 0.0)

            wv8_sb = cst.tile([128, 3, KT, 4 * D_HEAD], f8)
            nc.sync.dma_start(
                out=wv8_sb,
                in_=wv8_in.ap().rearrange("v (kt p) n -> p v kt n", p=128))
            wq8_sb = cst.tile([128, KT, 4 * D_HEAD], f8)
            wout_sb = cst.tile([128, NKT, D_MODEL], bf16)
            for dst, src in ((wq8_sb, wq8_in), (wout_sb, wout_in)):
                nc.sync.dma_start(
                    out=dst, in_=src.ap().rearrange("(kt p) n -> p kt n", p=128))
            bvp_bc = cst.tile([128, 4 * D_HEAD], f32)
            bq_bc = cst.tile([128, 4 * D_HEAD], f32)
            nc.sync.dma_start(out=bvp_bc, in_=bcast_ap(bvp_in))
            nc.sync.dma_start(out=bq_bc, in_=bcast_ap(bq_in))

            dt_sb = cst.tile([128, HPC, C], f32)
            nc.sync.dma_start(out=dt_sb, in_=dt_in.ap().rearrange("h b a -> b h a"))
            lam_bc = cst.tile([128, HPC * C], bf16)
            nc.sync.dma_start(out=lam_bc, in_=bcast_ap(lamc_in))
            pv = {}
            for name, src in (("mus", mus_in), ("omsig", omsig_in),
                              ("ratio", ratio_in), ("gamc", gamc_in),
                              ("epss", epss_in)):
                t = cst.tile([128, HPC], f32, name=f"pv_{name}", tag=f"pv_{name}")
                nc.sync.dma_start(out=t, in_=src.ap().rearrange("h p -> p h"))
                pv[name] = t

            outw_bc = cst.tile([128, D_MODEL], f32)
            outb_bc = cst.tile([128, D_MODEL], f32)
            nc.sync.dma_start(out=outw_bc, in_=bcast_ap(outw_in))
            nc.sync.dma_start(out=outb_bc, in_=bcast_ap(outb_in))

            for rep in range(reps):
                # ---- main loop: software-pipelined, batches interleaved.
                S_til = [None, None]
                khat2_prev = [None, None]

                def stage_a(t, b, tt):
                    xq_t = x8p.tile([128, 3, KT, C], f8, tag="xq",
                                    name=f"xq{rep}_{t}")
                    nc.sync.dma_start(out=xq_t, in_=xq_in[t])
                    if tt == 0:
                        S_til[b] = st.tile([128, HPC, D_HEAD + 1], bf16, tag="S",
                                           name=f"Sini{rep}_{t}")
                        nc.vector.memset(S_til[b][:], 0.0)

                    # v/p projection: 3 residual-compensated fp8 passes, PSUM = 32*vp
                    ps_vp = pproj.tile([128, 4 * D_HEAD], f32, tag="proj",
                                       name=f"psvp{rep}_{t}")
                    first = True
                    for v in range(3):
                        for kp in range(KT // 2):
                            for n in range(2):
                                ns = slice(n * 256, (n + 1) * 256)
                                nc.tensor.matmul(
                                    ps_vp[:, ns],
                                    xq_t[:, v, 2 * kp:2 * kp + 2, :],
                                    wv8_sb[:, v, 2 * kp:2 * kp + 2, ns],
                                    start=first,
                                    stop=(v == 2 and kp == KT // 2 - 1
                                          and n == 1),
                                    perf_mode=DR)
                                first = False
                    ps_qk = pproj.tile([128, 4 * D_HEAD], f32, tag="proj",
                                       name=f"psqk{rep}_{t}")
                    for kp in range(KT // 2):
                        for n in range(2):
                            ns = slice(n * 256, (n + 1) * 256)
                            nc.tensor.matmul(
                                ps_qk[:, ns],
                                xq_t[:, 0, 2 * kp:2 * kp + 2, :],
                                wq8_sb[:, 2 * kp:2 * kp + 2, ns],
                                start=(kp == 0 and n == 0),
                                stop=(kp == KT // 2 - 1 and n == 1),
                                perf_mode=DR)

                    v_aug = wk.tile([128, HPC, D_HEAD + 1], bf16, tag="vaug",
                                    name=f"vaug{rep}_{t}")
                    nc.vector.scalar_tensor_tensor(
                        out=v_aug[:, :, 0:D_HEAD],
                        in0=ps_vp[:, 0:2 * D_HEAD].rearrange(
                            "p (h d) -> p h d", h=HPC),
                        scalar=1.0 / S_QK,
                        in1=bvp_bc[:, 0:2 * D_HEAD].rearrange(
                            "p (h d) -> p h d", h=HPC),
                        op0=Alu.mult, op1=Alu.add)
                    nc.vector.memset(v_aug[:, :, D_HEAD:D_HEAD + 1], 1.0)

                    pb = wk.tile([128, 2 * D_HEAD], bf16, tag="pb",
                                 name=f"pb{rep}_{t}")
                    nc.vector.scalar_tensor_tensor(
                        out=pb[:], in0=ps_vp[:, 2 * D_HEAD:4 * D_HEAD],
                        scalar=1.0 / S_QK,
                        in1=bvp_bc[:, 2 * D_HEAD:4 * D_HEAD],
                        op0=Alu.mult, op1=Alu.add)
                    emp = wk.tile([128, 2 * D_HEAD], bf16, tag="emp",
                                  name=f"emp{rep}_{t}")
                    nc.scalar.activation(out=emp[:], in_=pb[:], func=Act.Exp,
                                         scale=-1.0)
                    nc.vector.tensor_scalar_add(out=emp[:], in0=emp[:],
                                                scalar1=1.0)
                    sig_p = wk.tile([128, 2 * D_HEAD], bf16, tag="sigp",
                                    name=f"sigp{rep}_{t}")
                    with nc.allow_low_precision(reason="sigmoid in [0,1]"):
                        nc.vector.reciprocal(out=sig_p[:], in_=emp[:])
                    silu_p = wk.tile([128, 2 * D_HEAD], bf16, tag="silup",
                                     name=f"silup{rep}_{t}")
                    nc.vector.tensor_mul(silu_p[:], pb[:], sig_p[:])

                    qb = wk.tile([128, 4 * D_HEAD], f32, tag="qb",
                                 name=f"qb{rep}_{t}")
                    nc.vector.scalar_tensor_tensor(
                        out=qb[:], in0=ps_qk[:], scalar=1.0 / S_QK,
                        in1=bq_bc[:], op0=Alu.mult, op1=Alu.add)

                    # A = [qhat | ktil]; one DMA-xbar transpose gives qk4T
                    A = wk.tile([128, 4 * D_HEAD], bf16, tag="A",
                                name=f"A{rep}_{t}")
                    zq = sm.tile([128, HPC], f32, tag="zq",
                                 name=f"zq{rep}_{t}")
                    expk = wk.tile([128, 2 * D_HEAD], bf16, tag="expk",
                                   name=f"expk{rep}_{t}")
                    zk = sm.tile([128, HPC], f32, tag="zk",
                                 name=f"zk{rep}_{t}")
                    for h in range(HPC):
                        hs = slice(h * D_HEAD, (h + 1) * D_HEAD)
                        ks = slice(2 * D_HEAD + h * D_HEAD,
                                   2 * D_HEAD + (h + 1) * D_HEAD)
                        nc.scalar.activation(out=A[:, hs],
                                             in_=qb[:, hs],
                                             func=Act.Exp,
                                             accum_out=zq[:, h:h + 1])
                        nc.scalar.activation(out=expk[:, hs],
                                             in_=qb[:, ks],
                                             func=Act.Exp,
                                             accum_out=zk[:, h:h + 1])
                    khat2 = wk.tile([128, 2 * D_HEAD], bf16, tag="khat2",
                                    name=f"khat2{rep}_{t}")
                    rzk = sm.tile([128, HPC], f32, tag="rzk",
                                  name=f"rzk{rep}_{t}")
                    nc.vector.reciprocal(out=rzk[:], in_=zk[:])
                    for h in range(HPC):
                        hs = slice(h * D_HEAD, (h + 1) * D_HEAD)
                        nc.vector.tensor_scalar(
                            out=khat2[:, hs], in0=expk[:, hs],
                            scalar1=rzk[:, h:h + 1],
                            scalar2=pv["omsig"][:, h:h + 1],
                            op0=Alu.mult, op1=Alu.mult)

                    kprev_ps = ptp.tile([128, 2 * D_HEAD], f32, tag="tp",
                                        name=f"kprev{rep}_{t}")
                    nc.tensor.matmul(kprev_ps[:], mshift[:], khat2[:],
                                     start=True, stop=False)
                    prev = khat2_prev[b] if tt > 0 else zero_k
                    nc.tensor.matmul(kprev_ps[:], ecorner[:], prev[:],
                                     start=False, stop=True)
                    khat2_prev[b] = khat2
                    kmu = wk.tile([128, 2 * D_HEAD], bf16, tag="kmu",
                                  name=f"kmu{rep}_{t}")
                    for h in range(HPC):
                        hs = slice(h * D_HEAD, (h + 1) * D_HEAD)
                        ats = slice(2 * D_HEAD + h * D_HEAD,
                                    2 * D_HEAD + (h + 1) * D_HEAD)
                        nc.vector.scalar_tensor_tensor(
                            out=A[:, ats], in0=kprev_ps[:, hs],
                            scalar=pv["ratio"][:, h:h + 1], in1=khat2[:, hs],
                            op0=Alu.mult, op1=Alu.add)
                        nc.vector.tensor_scalar_mul(
                            out=kmu[:, hs], in0=A[:, ats],
                            scalar1=pv["mus"][:, h:h + 1])

                    qk4T = wk.tile([128, 4, 128], bf16, tag="qk4T",
                                   name=f"qk4T{rep}_{t}")
                    nc.sync.dma_start_transpose(out=qk4T[:], in_=A[:])
                    qlam = wk.tile([128, HPC * 128], bf16, tag="qlam",
                                   name=f"qlam{rep}_{t}")
                    nc.vector.tensor_tensor(
                        out=qlam[:], in0=qk4T[:, 0:2, :].rearrange(
                            "p h d -> p (h d)"),
                        in1=lam_bc[:], op=Alu.mult)
                    return dict(qk4T=qk4T, qlam=qlam, v_aug=v_aug, kmu=kmu,
                                silu_p=silu_p, zq=zq, b=b)

                def stage_b(t, ctx):
                    b = ctx["b"]
                    qk4T, qlam = ctx["qk4T"], ctx["qlam"]
                    v_aug, kmu = ctx["v_aug"], ctx["kmu"]
                    silu_p, zq = ctx["silu_p"], ctx["zq"]
                    po = wk.tile([128, 2 * D_HEAD], bf16, tag="po",
                                 name=f"po{rep}_{t}")
                    o_ps = pacc.tile([128, HPC, D_HEAD + 1], f32, tag="acc",
                                     name=f"ops{rep}_{t}")
                    s_ps = pacc.tile([128, HPC, D_HEAD + 1], f32, tag="acc",
                                     name=f"sps{rep}_{t}")
                    S_new = st.tile([128, HPC, D_HEAD + 1], bf16, tag="S",
                                    name=f"Snew{rep}_{t}")
                    at_ps = pacc.tile([128, HPC, 128], f32, tag="acc",
                                      name=f"atps{rep}_{t}")
                    for h in range(HPC):
                        nc.tensor.matmul(at_ps[:, h, :], qk4T[:, 2 + h, :],
                                         qk4T[:, h, :], start=True, stop=True)
                    atm = wk.tile([128, HPC, 128], bf16, tag="atm",
                                  name=f"atm{rep}_{t}")
                    nc.vector.tensor_mul(atm[:], at_ps[:], dt_sb[:])
                    for h in range(HPC):
                        hs = slice(h * D_HEAD, (h + 1) * D_HEAD)
                        nc.tensor.matmul(o_ps[:, h, :], atm[:, h, :],
                                         v_aug[:, h, :],
                                         start=True, stop=False)
                        nc.tensor.matmul(o_ps[:, h, :], qlam[:, hs],
                                         S_til[b][:, h, :],
                                         start=False, stop=True)
                        nc.tensor.matmul(s_ps[:, h, :], kmu[:, hs],
                                         v_aug[:, h, :], start=True, stop=True)
                    zq_eps = sm.tile([128, HPC], f32, tag="zqe",
                                     name=f"zqe{rep}_{t}")
                    nc.vector.tensor_mul(zq_eps[:], zq[:], pv["epss"][:])
                    den = sm.tile([128, HPC], f32, tag="den",
                                  name=f"den{rep}_{t}")
                    nc.vector.tensor_add(
                        den[:], zq_eps[:],
                        o_ps[:, :, D_HEAD:D_HEAD + 1].rearrange(
                            "p h one -> p (h one)"))
                    rden = sm.tile([128, HPC], f32, tag="rden",
                                   name=f"rden{rep}_{t}")
                    nc.vector.reciprocal(out=rden[:], in_=den[:])
                    for h in range(HPC):
                        hs = slice(h * D_HEAD, (h + 1) * D_HEAD)
                        nc.vector.scalar_tensor_tensor(
                            out=po[:, hs], in0=o_ps[:, h, 0:D_HEAD],
                            scalar=rden[:, h:h + 1], in1=silu_p[:, hs],
                            op0=Alu.mult, op1=Alu.mult)
                        nc.vector.scalar_tensor_tensor(
                            out=S_new[:, h, :], in0=S_til[b][:, h, :],
                            scalar=pv["gamc"][:, h:h + 1], in1=s_ps[:, h, :],
                            op0=Alu.mult, op1=Alu.add)
                    S_til[b] = S_new

                    rb, cs = t // (RB // C), t % (RB // C)
                    po_ps = ptp.tile([128, HPC, 128], bf16, tag="tp",
                                     name=f"pops{rep}_{t}")
                    for h in range(HPC):
                        hs = slice(h * D_HEAD, (h + 1) * D_HEAD)
                        nc.tensor.transpose(po_ps[:, h, :], po[:, hs],
                                            identb[:])
                    poT = wk.tile([128, HPC, 128], bf16, tag="poT",
                                  name=f"poT{rep}_{t}")
                    nc.scalar.activation(out=poT[:], in_=po_ps[:],
                                         func=Act.Copy)
                    nc.sync.dma_start(out=pot_dram[rep % nex, rb, cs],
                                      in_=poT[:])

                pex = potex_dram[rep % nex]
                pin = pot_dram[rep % nex]

                its = [(b * TPB + tt, b, tt)
                       for tt in range(TPB) for b in range(B)]
                pend = []
                DEPTH = 2
                for i in range(len(its)):
                    pend.append((its[i][0], stage_a(*its[i])))
                    if len(pend) > DEPTH:
                        tb, cb = pend.pop(0)
                        stage_b(tb, cb)
                for tb, cb in pend:
                    stage_b(tb, cb)
                if no_collective:
                    nc.sync.dma_start(out=pex, in_=pin)
                else:
                    nc.gpsimd.collective_compute(
                        "AllToAll", Alu.bypass,
                        replica_groups=[list(range(N_CORES))],
                        ins=[pin], outs=[pex])

                # ---- out proj + final LN (two groups) ----
                for grp in range(2):
                    zts = []
                    mvf = sp.tile([128, 2, 2], f32, tag="mvf",
                                  name=f"mvf{rep}_{grp}")
                    for ti in range(2):
                        t = grp * 2 + ti
                        pox = wk2.tile([128, NKT, 128], bf16, tag="pox",
                                       name=f"pox{rep}_{t}")
                        nc.sync.dma_start(
                            out=pox[:].rearrange("p (s h) d -> p s h d",
                                                 s=N_CORES),
                            in_=pex[:, t].rearrange("s p h d -> p s h d"))
                        zr_t = zrp.tile([128, D_MODEL], f32, tag=f"zr{t}",
                                        name=f"zr{rep}_{t}")
                        for n in range(2):
                            ns = slice(n * 512, (n + 1) * 512)
                            z_ps = pproj.tile([128, 512], f32, tag="proj",
                                              name=f"zps{rep}_{t}_{n}")
                            for kt in range(NKT):
                                nc.tensor.matmul(z_ps[:], pox[:, kt, :],
                                                 wout_sb[:, kt, ns],
                                                 start=(kt == 0),
                                                 stop=(kt == NKT - 1))
                            nc.scalar.activation(out=zr_t[:, ns],
                                                 in_=z_ps[:], func=Act.Copy)
                        zts.append(zr_t)
                        stats = sm.tile([128, 2, 6], f32, tag="stats",
                                        name=f"st{rep}_{t}")
                        for i in range(2):
                            nc.vector.bn_stats(out=stats[:, i, :],
                                               in_=zr_t[:, i * 512:(i + 1) * 512])
                        nc.vector.bn_aggr(out=mvf[:, ti, :], in_=stats[:])
                    lnf = sp.tile([128, 2], f32, tag="lnf",
                                  name=f"lnf{rep}_{grp}")
                    nc.scalar.activation(out=lnf[:], in_=mvf[:, :, 1],
                                         func=Act.Ln, bias=eps_t[:])
                    rstdf = sp.tile([128, 2], f32, tag="rstdf",
                                    name=f"rstdf{rep}_{grp}")
                    nc.scalar.activation(out=rstdf[:], in_=lnf[:],
                                         func=Act.Exp, scale=-0.5)
                    for ti in range(2):
                        t = grp * 2 + ti
                        o_t = wk2.tile([128, D_MODEL], f32, tag="y",
                                       name=f"y{rep}_{t}")
                        nc.vector.tensor_scalar(
                            out=o_t[:], in0=zts[ti][:],
                            scalar1=mvf[:, ti, 0:1],
                            scalar2=rstdf[:, ti:ti + 1], op0=Alu.subtract,
                            op1=Alu.mult)
                        nc.vector.tensor_mul(o_t[:], o_t[:], outw_bc[:])
                        nc.vector.tensor_add(o_t[:], o_t[:], outb_bc[:])
                        nc.sync.dma_start(
                            out=out_ext[rep % n_out, t * C:(t + 1) * C, :],
                            in_=o_t[:])

    nc.compile()
    return nc


def prepare_in_maps(inputs: dict):
    """Host-side: fold LN affine params into weights, pre-normalize x,
    slice per core, build per-head decay constants."""
    import ml_dtypes

    def _bf(arr):
        return np.ascontiguousarray(np.asarray(arr).astype(ml_dtypes.bfloat16))

    def _f8(arr):
        return np.ascontiguousarray(
            np.asarray(arr).astype(ml_dtypes.float8_e4m3))

    x = np.ascontiguousarray(np.asarray(inputs["x"], np.float32)
                             .reshape(ROWS, D_MODEL))
    W_in = np.asarray(inputs["W_in"], np.float32)
    W_out = np.asarray(inputs["W_out"], np.float32)
    Wq = np.asarray(inputs["Wq"], np.float32)
    Wk = np.asarray(inputs["Wk"], np.float32)
    bq = np.asarray(inputs["bq"], np.float32)
    bk = np.asarray(inputs["bk"], np.float32)
    in_w = np.asarray(inputs["in_ln_w"], np.float32)
    in_b = np.asarray(inputs["in_ln_b"], np.float32)
    q_w = np.asarray(inputs["q_ln_w"], np.float32)
    q_b = np.asarray(inputs["q_ln_b"], np.float32)
    k_w = np.asarray(inputs["k_ln_w"], np.float32)
    k_b = np.asarray(inputs["k_ln_b"], np.float32)
    outw = np.asarray(inputs["out_ln_w"], np.float32)
    outb = np.asarray(inputs["out_ln_b"], np.float32)
    smear = np.asarray(inputs["smear_factor"], np.float32)
    log_scale = np.asarray(inputs["log_scale"], np.float32)

    Wvp_f = W_in * in_w[:, None]
    bvp_f = in_b @ W_in
    Wq_f = Wq * q_w[:, None]
    bq_f = bq + q_b @ Wq
    Wk_f = Wk * k_w[:, None]
    bk_f = bk + k_b @ Wk

    h2 = HEADS // 2
    slopes = np.concatenate([2.0 ** np.linspace(0.0, -8.0, h2),
                             np.zeros(HEADS - h2)]).astype(np.float64)
    sigm = 1.0 / (1.0 + np.exp(-smear.astype(np.float64)))
    s = np.exp(log_scale.astype(np.float64))

    a = np.arange(C)
    diff = a[:, None] - a[None, :]          # i - j
    # pre-normalized x-hat in per-row-tile contiguous layout
    mu = x.mean(1, keepdims=True)
    var = x.var(1, keepdims=True)
    xh = (x - mu) / np.sqrt(var + LN_EPS)
    xh_t = np.ascontiguousarray(
        xh.T.reshape(KT, 128, NT, C).transpose(2, 1, 0, 3))
    # packed fp8 streams: x8 = fp8(xh), x8c = fp8(xh/4), r8 = fp8(16*(xh-x8))
    x8 = _f8(xh_t)
    x8c = _f8(xh_t / 4.0)
    r8 = _f8(16.0 * (xh_t - x8.astype(np.float32)))
    xq = np.ascontiguousarray(
        np.stack([x8, x8c, r8], axis=2))     # [NT, 128, 3, KT, C]
    in_maps = []
    for c in range(N_CORES):
        heads = [HPC * c + i for i in range(HPC)]
        vcols = np.concatenate(
            [np.arange(h * D_HEAD, (h + 1) * D_HEAD) for h in heads])
        pcols = vcols + D_EXP
        dts, lams, muss, omsigs, ratios, gamcs, epsss = [], [], [], [], [], [], []
        for h in heads:
            lg = -slopes[h]                  # log gamma
            D = np.where(diff >= 0, np.exp(lg * diff), 0.0)   # [i, j]
            dts.append(D.T.astype(np.float32))                # [j, i]
            lams.append(np.exp(lg * (a + 1)).astype(np.float32))
            muss.append(np.exp(lg * (C - 1 - a)).astype(np.float32))
            omsigs.append(np.full(C, 1.0 - sigm[h], np.float32))
            ratios.append(np.full(C, sigm[h] / (1.0 - sigm[h]), np.float32))
            gamcs.append(np.full(C, np.exp(lg * C), np.float32))
            epsss.append(np.full(C, 1e-5 * s[h] * s[h], np.float32))
        wvp_c = np.ascontiguousarray(
            np.concatenate([Wvp_f[:, vcols], Wvp_f[:, pcols]], axis=1))
        bvp_c = np.concatenate([bvp_f[vcols], bvp_f[pcols]]).astype(np.float32)
        wq_c = np.concatenate([Wq_f[:, vcols], Wk_f[:, vcols]], axis=1)
        bq_c = np.concatenate([bq_f[vcols], bk_f[vcols]]).astype(np.float32)
        # residual-compensated fp8 v/p weights
        wvA = _f8(S_QK * wvp_c)
        epsA = wvA.astype(np.float32) - S_QK * wvp_c
        wvB = _f8(-4.0 * epsA)
        wvC = _f8(2.0 * wvp_c)
        in_maps.append({
            "xq": xq,
            "wv8": np.ascontiguousarray(np.stack([wvA, wvB, wvC])),
            "bvp": bvp_c,
            "wq8": _f8(S_QK * wq_c),
            "bq": bq_c,
            "wout": _bf(W_out),
            "outw": outw, "outb": outb,
            "dtmask": np.stack(dts),
            "lamc": _bf(np.concatenate(lams)),
            "mus": np.stack(muss),
            "omsig": np.stack(omsigs),
            "ratio": np.stack(ratios),
            "gamc": np.stack(gamcs),
            "epss": np.stack(epsss),
        })
    return in_maps


DEFAULT_MM_DT = "bf16"

_CACHED = {}


def _get_runner(mm_dt=None, reps=1):
    if mm_dt is None:
        mm_dt = DEFAULT_MM_DT
    key = (mm_dt, reps)
    if key not in _CACHED:
        nc = build_kernel(mm_dt=mm_dt, reps=reps)
        _CACHED[key] = nc
    return _CACHED[key]


def kernel(**inputs) -> np.ndarray:
    nc = _get_runner()
    in_maps = prepare_in_maps(inputs)
    from concourse.bass_utils import run_bass_kernel_spmd
    res = run_bass_kernel_spmd(nc, in_maps, list(range(N_CORES)))
    out = np.concatenate([res.results[c]["out"][0] for c in range(N_CORES)],
                         axis=0)
    return out.reshape(B, L, D_MODEL)


# revision 18
# speedup vs baseline: 1.1862x; 1.1862x over previous
"""Trainium2 Bass kernel for nn_Block_3539053052091 (hedgehog-style linear
attention block with ALiBi-decay mask, smeared keys, and sandwich layernorms).

Strategy (8 NeuronCores):
  - heads sharded: core c owns heads {2c, 2c+1} for both batches.
  - the host ships x-hat = LN(x) (pre-normalized, affine folded into the
    weights) in bf16 for the v/p projection and in fp8-e4m3 for the q/k
    projection, which runs as DoubleRow fp8 matmuls (2 k-planes per pass);
    per-column biases are added with DVE ops from partition-broadcast tiles.
  - q-hat / k-til transposes ride the DMA xbar (dma_start_transpose) instead
    of the PE; silu(p) is a single ACT Silu op.
  - the causal decayed attention is a chunked linear attention (chunk = 128
    rows) in bf16 with f32 PSUM; the q-softmax normalizer and the 1/s scales
    cancel through the attention row-normalizer; the reference's +1e-5 eps is
    reproduced exactly by adding 1e-5 * s^2 * zq_i to the denominator.
  - each core ships its po = silu(p) * o head-slice (bf16) through one
    AllToAll; it then computes z = po_rows @ W_out for its 512 rows,
    layernorms, and outputs; the host concatenates the 8 slices.
"""

import numpy as np

import concourse.bass as bass
import concourse.mybir as mybir
import concourse.tile as tile
from concourse import bacc

f32 = mybir.dt.float32
bf16 = mybir.dt.bfloat16
f8 = mybir.dt.float8e4

N_CORES = 8
B = 2
L = 2048
D_MODEL = 1024
HEADS = 16
EXP = 2
D_EXP = D_MODEL * EXP          # 2048
D_HEAD = D_EXP // HEADS        # 128
HPC = HEADS // N_CORES         # heads per core = 2
C = 128                        # chunk (= row tile) size
ROWS = B * L                   # 4096 flattened rows
NT = ROWS // C                 # 32 row tiles
TPB = L // C                   # 16 tiles per batch
KT = D_MODEL // 128            # 8 contraction tiles
NKT = D_EXP // 128             # 16 k-tiles for the output projection
LN_EPS = 1e-5
RB = ROWS // N_CORES           # 512 rows per core after the exchange
S_QK = 32.0                    # fp8 weight scale for the q/k projection

Act = mybir.ActivationFunctionType
Alu = mybir.AluOpType
DR = mybir.MatmulPerfMode.DoubleRow


def build_kernel(mm_dt: str = "bf16", reps: int = 1, no_collective: bool = False,
                 ring_out: bool = False):
    """ring_out=True: write outputs into a 2-slot ring (out shape [2, ...])
    so kernels with different rep counts have identical external I/O —
    needed for clean repeat-slope timing through the axon tunnel."""
    nc = bacc.Bacc("TRN2", target_bir_lowering=False, debug=False,
                   num_devices=N_CORES)

    # packed fp8 x-hat streams: plane 0 = fp8(xh), 1 = fp8(xh/4), 2 = fp8(16*(xh-x8))
    xq_in = nc.dram_tensor("xq", [NT, 128, 3, KT, C], f8, kind="ExternalInput")
    # packed fp8 v/p weights: plane 0 = fp8(32*W), 1 = fp8(-4*epsA), 2 = fp8(2*W)
    wv8_in = nc.dram_tensor("wv8", [3, D_MODEL, 4 * D_HEAD], f8, kind="ExternalInput")
    wq8_in = nc.dram_tensor("wq8", [D_MODEL, 4 * D_HEAD], f8, kind="ExternalInput")
    bvp_in = nc.dram_tensor("bvp32", [4 * D_HEAD], bf16, kind="ExternalInput")
    bq_in = nc.dram_tensor("bq32", [4 * D_HEAD], bf16, kind="ExternalInput")
    wout_in = nc.dram_tensor("wout", [D_EXP, D_MODEL], bf16, kind="ExternalInput")
    outw_in = nc.dram_tensor("outw", [D_MODEL], f32, kind="ExternalInput")
    outb_in = nc.dram_tensor("outb", [D_MODEL], f32, kind="ExternalInput")
    dt_in = nc.dram_tensor("dtmask", [HPC, C, C], f32, kind="ExternalInput")
    lamc_in = nc.dram_tensor("lamc", [HPC * C], bf16, kind="ExternalInput")
    # per-(row, head) constants, shipped as [HPC, C] -> loaded [C, HPC]
    mus_in = nc.dram_tensor("mus", [HPC, C], f32, kind="ExternalInput")
    omsig_in = nc.dram_tensor("omsig", [HPC, C], f32, kind="ExternalInput")
    ratio_in = nc.dram_tensor("ratio", [HPC, C], f32, kind="ExternalInput")
    gamc_in = nc.dram_tensor("gamc", [HPC, C], f32, kind="ExternalInput")
    epss_in = nc.dram_tensor("epss", [HPC, C], f32, kind="ExternalInput")

    n_out = min(reps, 2) if ring_out else reps
    out_ext = nc.dram_tensor("out", [n_out, RB, D_MODEL], f32,
                             kind="ExternalOutput")
    nex = 2 if reps > 1 else 1
    pot_dram = nc.dram_tensor("pot", [nex, N_CORES, RB // C, 128, HPC, C],
                              bf16)
    potex_dram = nc.dram_tensor("potex", [nex, N_CORES, RB // C, 128, HPC, C],
                                bf16)

    def bcast_ap(handle, parts=128):
        ap = handle.ap()
        return bass.AP(tensor=ap.tensor, offset=ap.offset,
                       ap=[[0, parts]] + list(ap.ap))

    with tile.TileContext(nc) as tc:
        with (
            tc.tile_pool(name="const", bufs=1) as cst,
            tc.tile_pool(name="zrp", bufs=1) as zrp,
            tc.tile_pool(name="work", bufs=4) as wk,
            tc.tile_pool(name="work2", bufs=2) as wk2,
            tc.tile_pool(name="x8p", bufs=3) as x8p,
            tc.tile_pool(name="small", bufs=4) as sm,
            tc.tile_pool(name="state", bufs=6) as st,
            tc.tile_pool(name="statp", bufs=1) as sp,
            tc.tile_pool(name="pproj", bufs=4, space="PSUM") as pproj,
            tc.tile_pool(name="ptp", bufs=2, space="PSUM") as ptp,
            tc.tile_pool(name="pacc", bufs=2, space="PSUM") as pacc,
        ):
            # ---- constants ----
            # shift matrices for the smeared-key row shift:
            # mshift[j', j'+1] = 1 (out row j <- khat2 row j-1)
            mshift = cst.tile([128, 128], bf16)
            nc.gpsimd.memset(mshift[:], 0.0)
            nc.gpsimd.affine_select(
                out=mshift[:], in_=mshift[:],
                compare_op=Alu.not_equal, fill=1.0, base=1,
                pattern=[[-1, 128]], channel_multiplier=1)
            # ecorner[127, 0] = 1 (out row 0 <- prev tile's row 127)
            ecorner = cst.tile([128, 128], bf16)
            nc.gpsimd.memset(ecorner[:], 0.0)
            nc.gpsimd.affine_select(
                out=ecorner[:], in_=ecorner[:],
                compare_op=Alu.not_equal, fill=1.0, base=-127,
                pattern=[[-1, 128]], channel_multiplier=1)
            eps_t = cst.tile([128, 1], f32)
            nc.vector.memset(eps_t[:], LN_EPS)
            zero_k = cst.tile([128, 2 * D_HEAD], bf16)
            nc.vector.memset(zero_k[:], 0.0)

            wv8_sb = cst.tile([128, 3, KT, 4 * D_HEAD], f8)
            nc.sync.dma_start(
                out=wv8_sb,
                in_=wv8_in.ap().rearrange("v (kt p) n -> p v kt n", p=128))
            wq8_sb = cst.tile([128, KT, 4 * D_HEAD], f8)
            wout_sb = cst.tile([128, NKT, D_MODEL], bf16)
            for dst, src in ((wq8_sb, wq8_in), (wout_sb, wout_in)):
                nc.sync.dma_start(
                    out=dst, in_=src.ap().rearrange("(kt p) n -> p kt n", p=128))
            ones1 = cst.tile([1, 128], bf16)
            nc.vector.memset(ones1[:], 1.0)
            bvp_r = cst.tile([1, 4 * D_HEAD], bf16)
            bq_r = cst.tile([1, 4 * D_HEAD], bf16)
            nc.sync.dma_start(out=bvp_r, in_=bvp_in.ap())
            nc.sync.dma_start(out=bq_r, in_=bq_in.ap())

            dt_sb = cst.tile([128, HPC, C], f32)
            nc.sync.dma_start(out=dt_sb, in_=dt_in.ap().rearrange("h b a -> b h a"))
            lam_bc = cst.tile([128, HPC * C], bf16)
            nc.sync.dma_start(out=lam_bc, in_=bcast_ap(lamc_in))
            pv = {}
            for name, src in (("mus", mus_in), ("omsig", omsig_in),
                              ("ratio", ratio_in), ("gamc", gamc_in),
                              ("epss", epss_in)):
                t = cst.tile([128, HPC], f32, name=f"pv_{name}", tag=f"pv_{name}")
                nc.sync.dma_start(out=t, in_=src.ap().rearrange("h p -> p h"))
                pv[name] = t

            outw_bc = cst.tile([128, D_MODEL], f32)
            outb_bc = cst.tile([128, D_MODEL], f32)
            nc.sync.dma_start(out=outw_bc, in_=bcast_ap(outw_in))
            nc.sync.dma_start(out=outb_bc, in_=bcast_ap(outb_in))

            for rep in range(reps):
                # ---- main loop: software-pipelined, batches interleaved.
                S_til = [None, None]
                khat2_prev = [None, None]

                def stage_a(t, b, tt):
                    xq_t = x8p.tile([128, 3, KT, C], f8, tag="xq",
                                    name=f"xq{rep}_{t}")
                    nc.sync.dma_start(out=xq_t, in_=xq_in[t])
                    if tt == 0:
                        S_til[b] = st.tile([128, HPC, D_HEAD + 1], bf16, tag="S",
                                           name=f"Sini{rep}_{t}")
                        nc.vector.memset(S_til[b][:], 0.0)

                    # v/p projection: 3 residual-compensated fp8 passes, PSUM = 32*vp
                    ps_vp = pproj.tile([128, 4 * D_HEAD], f32, tag="proj",
                                       name=f"psvp{rep}_{t}")
                    first = True
                    for v in range(3):
                        for kp in range(KT // 2):
                            for n in range(2):
                                ns = slice(n * 256, (n + 1) * 256)
                                nc.tensor.matmul(
                                    ps_vp[:, ns],
                                    xq_t[:, v, 2 * kp:2 * kp + 2, :],
                                    wv8_sb[:, v, 2 * kp:2 * kp + 2, ns],
                                    start=first, stop=False,
                                    perf_mode=DR)
                                first = False
                    nc.tensor.matmul(ps_vp[:], ones1[:], bvp_r[:],
                                     start=False, stop=True)
                    ps_qk = pproj.tile([128, 4 * D_HEAD], f32, tag="proj",
                                       name=f"psqk{rep}_{t}")
                    for kp in range(KT // 2):
                        for n in range(2):
                            ns = slice(n * 256, (n + 1) * 256)
                            nc.tensor.matmul(
                                ps_qk[:, ns],
                                xq_t[:, 0, 2 * kp:2 * kp + 2, :],
                                wq8_sb[:, 2 * kp:2 * kp + 2, ns],
                                start=(kp == 0 and n == 0), stop=False,
                                perf_mode=DR)
                    nc.tensor.matmul(ps_qk[:], ones1[:], bq_r[:],
                                     start=False, stop=True)

                    v_aug = wk.tile([128, HPC, D_HEAD + 1], bf16, tag="vaug",
                                    name=f"vaug{rep}_{t}")
                    nc.vector.tensor_scalar_mul(
                        out=v_aug[:, :, 0:D_HEAD],
                        in0=ps_vp[:, 0:2 * D_HEAD].rearrange(
                            "p (h d) -> p h d", h=HPC),
                        scalar1=1.0 / S_QK)
                    nc.vector.memset(v_aug[:, :, D_HEAD:D_HEAD + 1], 1.0)

                    emp = wk.tile([128, 2 * D_HEAD], bf16, tag="emp",
                                  name=f"emp{rep}_{t}")
                    nc.scalar.activation(out=emp[:],
                                         in_=ps_vp[:, 2 * D_HEAD:4 * D_HEAD],
                                         func=Act.Exp, scale=-1.0 / S_QK)
                    nc.vector.tensor_scalar_add(out=emp[:], in0=emp[:],
                                                scalar1=1.0)
                    sig_p = wk.tile([128, 2 * D_HEAD], bf16, tag="sigp",
                                    name=f"sigp{rep}_{t}")
                    with nc.allow_low_precision(reason="sigmoid in [0,1]"):
                        nc.vector.reciprocal(out=sig_p[:], in_=emp[:])
                    silu_p = wk.tile([128, 2 * D_HEAD], bf16, tag="silup",
                                     name=f"silup{rep}_{t}")
                    nc.vector.scalar_tensor_tensor(
                        out=silu_p[:], in0=ps_vp[:, 2 * D_HEAD:4 * D_HEAD],
                        scalar=1.0 / S_QK, in1=sig_p[:],
                        op0=Alu.mult, op1=Alu.mult)

                    # A = [qhat | ktil]; one DMA-xbar transpose gives qk4T
                    A = wk.tile([128, 4 * D_HEAD], bf16, tag="A",
                                name=f"A{rep}_{t}")
                    zq = sm.tile([128, HPC], f32, tag="zq",
                                 name=f"zq{rep}_{t}")
                    expk = wk.tile([128, 2 * D_HEAD], bf16, tag="expk",
                                   name=f"expk{rep}_{t}")
                    zk = sm.tile([128, HPC], f32, tag="zk",
                                 name=f"zk{rep}_{t}")
                    for h in range(HPC):
                        hs = slice(h * D_HEAD, (h + 1) * D_HEAD)
                        ks = slice(2 * D_HEAD + h * D_HEAD,
                                   2 * D_HEAD + (h + 1) * D_HEAD)
                        nc.scalar.activation(out=A[:, hs],
                                             in_=ps_qk[:, hs],
                                             func=Act.Exp, scale=1.0 / S_QK,
                                             accum_out=zq[:, h:h + 1])
                        nc.scalar.activation(out=expk[:, hs],
                                             in_=ps_qk[:, ks],
                                             func=Act.Exp, scale=1.0 / S_QK,
                                             accum_out=zk[:, h:h + 1])
                    khat2 = wk.tile([128, 2 * D_HEAD], bf16, tag="khat2",
                                    name=f"khat2{rep}_{t}")
                    rzk = sm.tile([128, HPC], f32, tag="rzk",
                                  name=f"rzk{rep}_{t}")
                    nc.vector.reciprocal(out=rzk[:], in_=zk[:])
                    for h in range(HPC):
                        hs = slice(h * D_HEAD, (h + 1) * D_HEAD)
                        nc.vector.tensor_scalar(
                            out=khat2[:, hs], in0=expk[:, hs],
                            scalar1=rzk[:, h:h + 1],
                            scalar2=pv["omsig"][:, h:h + 1],
                            op0=Alu.mult, op1=Alu.mult)

                    kprev_ps = ptp.tile([128, 2 * D_HEAD], f32, tag="tp",
                                        name=f"kprev{rep}_{t}")
                    nc.tensor.matmul(kprev_ps[:], mshift[:], khat2[:],
                                     start=True, stop=False)
                    prev = khat2_prev[b] if tt > 0 else zero_k
                    nc.tensor.matmul(kprev_ps[:], ecorner[:], prev[:],
                                     start=False, stop=True)
                    khat2_prev[b] = khat2
                    kmu = wk.tile([128, 2 * D_HEAD], bf16, tag="kmu",
                                  name=f"kmu{rep}_{t}")
                    for h in range(HPC):
                        hs = slice(h * D_HEAD, (h + 1) * D_HEAD)
                        ats = slice(2 * D_HEAD + h * D_HEAD,
                                    2 * D_HEAD + (h + 1) * D_HEAD)
                        nc.vector.scalar_tensor_tensor(
                            out=A[:, ats], in0=kprev_ps[:, hs],
                            scalar=pv["ratio"][:, h:h + 1], in1=khat2[:, hs],
                            op0=Alu.mult, op1=Alu.add)
                        nc.vector.tensor_scalar_mul(
                            out=kmu[:, hs], in0=A[:, ats],
                            scalar1=pv["mus"][:, h:h + 1])

                    qk4T = wk.tile([128, 4, 128], bf16, tag="qk4T",
                                   name=f"qk4T{rep}_{t}")
                    nc.sync.dma_start_transpose(out=qk4T[:], in_=A[:])
                    qlam = wk.tile([128, HPC * 128], bf16, tag="qlam",
                                   name=f"qlam{rep}_{t}")
                    nc.vector.tensor_tensor(
                        out=qlam[:], in0=qk4T[:, 0:2, :].rearrange(
                            "p h d -> p (h d)"),
                        in1=lam_bc[:], op=Alu.mult)
                    return dict(qk4T=qk4T, qlam=qlam, v_aug=v_aug, kmu=kmu,
                                silu_p=silu_p, zq=zq, b=b)

                def stage_b(t, ctx):
                    b = ctx["b"]
                    qk4T, qlam = ctx["qk4T"], ctx["qlam"]
                    v_aug, kmu = ctx["v_aug"], ctx["kmu"]
                    silu_p, zq = ctx["silu_p"], ctx["zq"]
                    po = wk.tile([128, 2 * D_HEAD], bf16, tag="po",
                                 name=f"po{rep}_{t}")
                    o_ps = pacc.tile([128, HPC, D_HEAD + 1], f32, tag="acc",
                                     name=f"ops{rep}_{t}")
                    s_ps = pacc.tile([128, HPC, D_HEAD + 1], f32, tag="acc",
                                     name=f"sps{rep}_{t}")
                    S_new = st.tile([128, HPC, D_HEAD + 1], bf16, tag="S",
                                    name=f"Snew{rep}_{t}")
                    at_ps = pacc.tile([128, HPC, 128], f32, tag="acc",
                                      name=f"atps{rep}_{t}")
                    for h in range(HPC):
                        nc.tensor.matmul(at_ps[:, h, :], qk4T[:, 2 + h, :],
                                         qk4T[:, h, :], start=True, stop=True)
                    atm = wk.tile([128, HPC, 128], bf16, tag="atm",
                                  name=f"atm{rep}_{t}")
                    nc.vector.tensor_mul(atm[:], at_ps[:], dt_sb[:])
                    for h in range(HPC):
                        hs = slice(h * D_HEAD, (h + 1) * D_HEAD)
                        nc.tensor.matmul(o_ps[:, h, :], atm[:, h, :],
                                         v_aug[:, h, :],
                                         start=True, stop=False)
                        nc.tensor.matmul(o_ps[:, h, :], qlam[:, hs],
                                         S_til[b][:, h, :],
                                         start=False, stop=True)
                        nc.tensor.matmul(s_ps[:, h, :], kmu[:, hs],
                                         v_aug[:, h, :], start=True, stop=True)
                    zq_eps = sm.tile([128, HPC], f32, tag="zqe",
                                     name=f"zqe{rep}_{t}")
                    nc.vector.tensor_mul(zq_eps[:], zq[:], pv["epss"][:])
                    den = sm.tile([128, HPC], f32, tag="den",
                                  name=f"den{rep}_{t}")
                    nc.vector.tensor_add(
                        den[:], zq_eps[:],
                        o_ps[:, :, D_HEAD:D_HEAD + 1].rearrange(
                            "p h one -> p (h one)"))
                    rden = sm.tile([128, HPC], f32, tag="rden",
                                   name=f"rden{rep}_{t}")
                    nc.vector.reciprocal(out=rden[:], in_=den[:])
                    for h in range(HPC):
                        hs = slice(h * D_HEAD, (h + 1) * D_HEAD)
                        nc.vector.scalar_tensor_tensor(
                            out=po[:, hs], in0=o_ps[:, h, 0:D_HEAD],
                            scalar=rden[:, h:h + 1], in1=silu_p[:, hs],
                            op0=Alu.mult, op1=Alu.mult)
                        nc.vector.scalar_tensor_tensor(
                            out=S_new[:, h, :], in0=S_til[b][:, h, :],
                            scalar=pv["gamc"][:, h:h + 1], in1=s_ps[:, h, :],
                            op0=Alu.mult, op1=Alu.add)
                    S_til[b] = S_new

                    rb, cs = t // (RB // C), t % (RB // C)
                    poT = wk.tile([128, HPC, 128], bf16, tag="poT",
                                  name=f"poT{rep}_{t}")
                    nc.sync.dma_start_transpose(out=poT[:], in_=po[:])
                    nc.sync.dma_start(out=pot_dram[rep % nex, rb, cs],
                                      in_=poT[:])

                pex = potex_dram[rep % nex]
                pin = pot_dram[rep % nex]

                its = [(b * TPB + tt, b, tt)
                       for tt in range(TPB) for b in range(B)]
                pend = []
                DEPTH = 2
                for i in range(len(its)):
                    pend.append((its[i][0], stage_a(*its[i])))
                    if len(pend) > DEPTH:
                        tb, cb = pend.pop(0)
                        stage_b(tb, cb)
                for tb, cb in pend:
                    stage_b(tb, cb)
                if no_collective:
                    nc.sync.dma_start(out=pex, in_=pin)
                else:
                    nc.gpsimd.collective_compute(
                        "AllToAll", Alu.bypass,
                        replica_groups=[list(range(N_CORES))],
                        ins=[pin], outs=[pex])

                # ---- out proj + final LN (two groups) ----
                for grp in range(2):
                    zts = []
                    mvf = sp.tile([128, 2, 2], f32, tag="mvf",
                                  name=f"mvf{rep}_{grp}")
                    for ti in range(2):
                        t = grp * 2 + ti
                        pox = wk2.tile([128, NKT, 128], bf16, tag="pox",
                                       name=f"pox{rep}_{t}")
                        nc.sync.dma_start(
                            out=pox[:].rearrange("p (s h) d -> p s h d",
                                                 s=N_CORES),
                            in_=pex[:, t].rearrange("s p h d -> p s h d"))
                        zr_t = zrp.tile([128, D_MODEL], f32, tag=f"zr{t}",
                                        name=f"zr{rep}_{t}")
                        for n in range(2):
                            ns = slice(n * 512, (n + 1) * 512)
                            z_ps = pproj.tile([128, 512], f32, tag="proj",
                                              name=f"zps{rep}_{t}_{n}")
                            for kt in range(NKT):
                                nc.tensor.matmul(z_ps[:], pox[:, kt, :],
                                                 wout_sb[:, kt, ns],
                                                 start=(kt == 0),
                                                 stop=(kt == NKT - 1))
                            nc.scalar.activation(out=zr_t[:, ns],
                                                 in_=z_ps[:], func=Act.Copy)
                        zts.append(zr_t)
                        stats = sm.tile([128, 2, 6], f32, tag="stats",
                                        name=f"st{rep}_{t}")
                        for i in range(2):
                            nc.vector.bn_stats(out=stats[:, i, :],
                                               in_=zr_t[:, i * 512:(i + 1) * 512])
                        nc.vector.bn_aggr(out=mvf[:, ti, :], in_=stats[:])
                    lnf = sp.tile([128, 2], f32, tag="lnf",
                                  name=f"lnf{rep}_{grp}")
                    nc.scalar.activation(out=lnf[:], in_=mvf[:, :, 1],
                                         func=Act.Ln, bias=eps_t[:])
                    rstdf = sp.tile([128, 2], f32, tag="rstdf",
                                    name=f"rstdf{rep}_{grp}")
                    nc.scalar.activation(out=rstdf[:], in_=lnf[:],
                                         func=Act.Exp, scale=-0.5)
                    for ti in range(2):
                        t = grp * 2 + ti
                        o_t = wk2.tile([128, D_MODEL], f32, tag="y",
                                       name=f"y{rep}_{t}")
                        nc.vector.tensor_scalar(
                            out=o_t[:], in0=zts[ti][:],
                            scalar1=mvf[:, ti, 0:1],
                            scalar2=rstdf[:, ti:ti + 1], op0=Alu.subtract,
                            op1=Alu.mult)
                        nc.vector.tensor_mul(o_t[:], o_t[:], outw_bc[:])
                        nc.vector.tensor_add(o_t[:], o_t[:], outb_bc[:])
                        nc.sync.dma_start(
                            out=out_ext[rep % n_out, t * C:(t + 1) * C, :],
                            in_=o_t[:])

    nc.compile()
    return nc


def prepare_in_maps(inputs: dict):
    """Host-side: fold LN affine params into weights, pre-normalize x,
    slice per core, build per-head decay constants."""
    import ml_dtypes

    def _bf(arr):
        return np.ascontiguousarray(np.asarray(arr).astype(ml_dtypes.bfloat16))

    def _f8(arr):
        return np.ascontiguousarray(
            np.asarray(arr).astype(ml_dtypes.float8_e4m3))

    x = np.ascontiguousarray(np.asarray(inputs["x"], np.float32)
                             .reshape(ROWS, D_MODEL))
    W_in = np.asarray(inputs["W_in"], np.float32)
    W_out = np.asarray(inputs["W_out"], np.float32)
    Wq = np.asarray(inputs["Wq"], np.float32)
    Wk = np.asarray(inputs["Wk"], np.float32)
    bq = np.asarray(inputs["bq"], np.float32)
    bk = np.asarray(inputs["bk"], np.float32)
    in_w = np.asarray(inputs["in_ln_w"], np.float32)
    in_b = np.asarray(inputs["in_ln_b"], np.float32)
    q_w = np.asarray(inputs["q_ln_w"], np.float32)
    q_b = np.asarray(inputs["q_ln_b"], np.float32)
    k_w = np.asarray(inputs["k_ln_w"], np.float32)
    k_b = np.asarray(inputs["k_ln_b"], np.float32)
    outw = np.asarray(inputs["out_ln_w"], np.float32)
    outb = np.asarray(inputs["out_ln_b"], np.float32)
    smear = np.asarray(inputs["smear_factor"], np.float32)
    log_scale = np.asarray(inputs["log_scale"], np.float32)

    Wvp_f = W_in * in_w[:, None]
    bvp_f = in_b @ W_in
    Wq_f = Wq * q_w[:, None]
    bq_f = bq + q_b @ Wq
    Wk_f = Wk * k_w[:, None]
    bk_f = bk + k_b @ Wk

    h2 = HEADS // 2
    slopes = np.concatenate([2.0 ** np.linspace(0.0, -8.0, h2),
                             np.zeros(HEADS - h2)]).astype(np.float64)
    sigm = 1.0 / (1.0 + np.exp(-smear.astype(np.float64)))
    s = np.exp(log_scale.astype(np.float64))

    a = np.arange(C)
    diff = a[:, None] - a[None, :]          # i - j
    # pre-normalized x-hat in per-row-tile contiguous layout
    mu = x.mean(1, keepdims=True)
    var = x.var(1, keepdims=True)
    xh = (x - mu) / np.sqrt(var + LN_EPS)
    xh_t = np.ascontiguousarray(
        xh.T.reshape(KT, 128, NT, C).transpose(2, 1, 0, 3))
    # packed fp8 streams: x8 = fp8(xh), x8c = fp8(xh/4), r8 = fp8(16*(xh-x8))
    x8 = _f8(xh_t)
    x8c = _f8(xh_t / 4.0)
    r8 = _f8(16.0 * (xh_t - x8.astype(np.float32)))
    xq = np.ascontiguousarray(
        np.stack([x8, x8c, r8], axis=2))     # [NT, 128, 3, KT, C]
    in_maps = []
    for c in range(N_CORES):
        heads = [HPC * c + i for i in range(HPC)]
        vcols = np.concatenate(
            [np.arange(h * D_HEAD, (h + 1) * D_HEAD) for h in heads])
        pcols = vcols + D_EXP
        dts, lams, muss, omsigs, ratios, gamcs, epsss = [], [], [], [], [], [], []
        for h in heads:
            lg = -slopes[h]                  # log gamma
            D = np.where(diff >= 0, np.exp(lg * diff), 0.0)   # [i, j]
            dts.append(D.T.astype(np.float32))                # [j, i]
            lams.append(np.exp(lg * (a + 1)).astype(np.float32))
            muss.append(np.exp(lg * (C - 1 - a)).astype(np.float32))
            omsigs.append(np.full(C, 1.0 - sigm[h], np.float32))
            ratios.append(np.full(C, sigm[h] / (1.0 - sigm[h]), np.float32))
            gamcs.append(np.full(C, np.exp(lg * C), np.float32))
            epsss.append(np.full(C, 1e-5 * s[h] * s[h], np.float32))
        wvp_c = np.ascontiguousarray(
            np.concatenate([Wvp_f[:, vcols], Wvp_f[:, pcols]], axis=1))
        bvp_c = np.concatenate([bvp_f[vcols], bvp_f[pcols]]).astype(np.float32)
        wq_c = np.concatenate([Wq_f[:, vcols], Wk_f[:, vcols]], axis=1)
        bq_c = np.concatenate([bq_f[vcols], bk_f[vcols]]).astype(np.float32)
        # residual-compensated fp8 v/p weights
        wvA = _f8(S_QK * wvp_c)
        epsA = wvA.astype(np.float32) - S_QK * wvp_c
        wvB = _f8(-4.0 * epsA)
        wvC = _f8(2.0 * wvp_c)
        in_maps.append({
            "xq": xq,
            "wv8": np.ascontiguousarray(np.stack([wvA, wvB, wvC])),
            "bvp32": _bf(S_QK * bvp_c),
            "wq8": _f8(S_QK * wq_c),
            "bq32": _bf(S_QK * bq_c),
            "wout": _bf(W_out),
            "outw": outw, "outb": outb,
            "dtmask": np.stack(dts),
            "lamc": _bf(np.concatenate(lams)),
            "mus": np.stack(muss),
            "omsig": np.stack(omsigs),
            "ratio": np.stack(ratios),
            "gamc": np.stack(gamcs),
            "epss": np.stack(epsss),
        })
    return in_maps


DEFAULT_MM_DT = "bf16"

_CACHED = {}


def _get_runner(mm_dt=None, reps=1):
    if mm_dt is None:
        mm_dt = DEFAULT_MM_DT
    key = (mm_dt, reps)
    if key not in _CACHED:
        nc = build_kernel(mm_dt=mm_dt, reps=reps)
        _CACHED[key] = nc
    return _CACHED[key]


def kernel(**inputs) -> np.ndarray:
    nc = _get_runner()
    in_maps = prepare_in_maps(inputs)
    from concourse.bass_utils import run_bass_kernel_spmd
    res = run_bass_kernel_spmd(nc, in_maps, list(range(N_CORES)))
    out = np.concatenate([res.results[c]["out"][0] for c in range(N_CORES)],
                         axis=0)
    return out.reshape(B, L, D_MODEL)
